# revision 2
# baseline (speedup 1.0000x reference)
"""BlockAttention prefill kernel for Trainium2, 8-core tensor-parallel.

Reference op (see problem): scatter K/V rows into paged caches, then
block-causal (staircase, block_length=32) attention over T=4096 tokens,
16 query heads / 4 KV heads (GQA), head_dim=128, fp32.

Sharding: pure tensor parallelism over heads. Core c computes query heads
{2c, 2c+1}, which share KV head c//2. Cache update is split so core 2j
produces the K-cache slice of KV head j and core 2j+1 the V-cache slice.

Per-core kernel layout (one SPMD Bass program, data differs per core):
  S_T[tk, q] = (K_tile)^T-style scores with q streaming (N=512 supertiles)
  exp on ACT in batches of <=3 k-tiles (one PSUM-wide activation)
  staircase masking applied multiplicatively after exp (fp16)
  PV uses P_T chunks as stationary operand; V carries an extra ones
  column so the softmax denominator accumulates in PSUM alongside O.
"""

import os
import numpy as np

T = 4096
H = 16
HKV = 4
D = 128
BL = 32
NUM_SLOTS = 8192
SCALE = 0.08838834764831845
NCORES = 8
QH = 2                    # query heads per core
QSUP = 512                # queries per supertile (fp32 matmul N max)
NSUP = T // QSUP          # 8
KTILE = 128
NKT = T // KTILE          # 32
KBATCH = 3                # k-tiles per exp batch (PSUM: 2*3 + 2 banks)
DV = D + 1                # V width incl. ones column

_PROG_CACHE = {}
LAST_RESULT = None


def _plan_cache(slot_mapping):
    """Coalesce the cache scatter into contiguous row-range copies.

    Returns segments (dst_start, src_start, n, from_new): from_new rows come
    from the new k/v rows, others pass through the input cache.
    """
    sm = np.asarray(slot_mapping).astype(np.int64)
    src_of = np.full(NUM_SLOTS, -1, np.int64)
    src_of[sm] = np.arange(sm.shape[0])
    segs = []
    r = 0
    while r < NUM_SLOTS:
        if src_of[r] < 0:
            r2 = r
            while r2 < NUM_SLOTS and src_of[r2] < 0:
                r2 += 1
            segs.append((r, r, r2 - r, False))
            r = r2
        else:
            r2 = r
            while r2 + 1 < NUM_SLOTS and src_of[r2 + 1] == src_of[r2] + 1:
                r2 += 1
            segs.append((r, int(src_of[r]), r2 - r + 1, True))
            r = r2 + 1
    return tuple(segs)


def _build_program(plan):
    import concourse.mybir as mybir
    from concourse import bacc
    from concourse.tile import TileContext

    f32 = mybir.dt.float32
    f32r = mybir.dt.float32r
    f16 = mybir.dt.float16
    EXP = mybir.ActivationFunctionType.Exp

    nc = bacc.Bacc("TRN2", target_bir_lowering=False, debug=False,
                   num_devices=NCORES)

    qT = nc.declare_dram_parameter("qT", [QH, 128, T], f32r, isOutput=False)
    kT = nc.declare_dram_parameter("kT", [128, T], f32r, isOutput=False)
    vp = nc.declare_dram_parameter("vp", [128, NKT * DV], f16, isOutput=False)
    mk = nc.declare_dram_parameter("mk", [128, 4 * QSUP], f16, isOutput=False)
    cin = nc.declare_dram_parameter("cin", [NUM_SLOTS, D], f32, isOutput=False)
    src = nc.declare_dram_parameter("src", [T, D], f32, isOutput=False)
    o_part = nc.declare_dram_parameter("o_part", [T, QH * D], f32, isOutput=True)
    cout = nc.declare_dram_parameter("cout", [NUM_SLOTS, D], f32, isOutput=True)

    with TileContext(nc) as tc:
        with tc.tile_pool(name="const", bufs=1) as cpool, \
             tc.tile_pool(name="work", bufs=3) as wpool, \
             tc.tile_pool(name="stp", bufs=2, space="PSUM") as stpool, \
             tc.tile_pool(name="opsum", bufs=1, space="PSUM") as opool, \
             tc.tile_pool(name="outp", bufs=4) as outpool:

            qT_sb = cpool.tile([128, QH * T], f32r, tag="qT_sb", name="qT_sb")
            kT_sb = cpool.tile([128, T], f32r, tag="kT_sb", name="kT_sb")
            vp_sb = cpool.tile([128, NKT * DV], f16, tag="vp_sb", name="vp_sb")
            mk_sb = cpool.tile([128, 4 * QSUP], f16, tag="mk_sb", name="mk_sb")

            # Loads, most urgent first (h0/s0 needs kT[0:512], qT h0 s0,
            # vp tiles 0..3, masks).
            nc.sync.dma_start(out=kT_sb[:, 0:512], in_=kT[:, 0:512])
            nc.sync.dma_start(out=qT_sb[:, 0:QSUP], in_=qT[0, :, 0:QSUP])
            nc.sync.dma_start(out=vp_sb[:, 0:8 * DV], in_=vp[:, 0:8 * DV])
            nc.sync.dma_start(out=mk_sb[:, :], in_=mk[:, :])
            for j in range(1, 8):
                nc.sync.dma_start(out=kT_sb[:, j * 512:(j + 1) * 512],
                                  in_=kT[:, j * 512:(j + 1) * 512])
            for j in range(1, 4):
                nc.sync.dma_start(out=vp_sb[:, j * 8 * DV:(j + 1) * 8 * DV],
                                  in_=vp[:, j * 8 * DV:(j + 1) * 8 * DV])
            for h in range(QH):
                for s in range(NSUP):
                    if h == 0 and s == 0:
                        continue
                    off = h * T + s * QSUP
                    nc.sync.dma_start(out=qT_sb[:, off:off + QSUP],
                                      in_=qT[h, :, s * QSUP:(s + 1) * QSUP])

            pending = [None]

            def flush():
                if pending[0] is None:
                    return
                batch, pt, o01, o23, s, nk, h, last = pending[0]
                pending[0] = None
                for j, ki in enumerate(batch):
                    for c in range(4):
                        ot = o01 if c < 2 else o23
                        col = (c % 2) * DV
                        nc.tensor.matmul(
                            ot[:, col:col + DV],
                            lhsT=pt[:, j * QSUP + c * 128:j * QSUP + (c + 1) * 128],
                            rhs=vp_sb[:, ki * DV:(ki + 1) * DV],
                            start=(ki == 0),
                            stop=(ki == nk - 1),
                        )
                if last:
                    for c in range(4):
                        ot = o01 if c < 2 else o23
                        col = (c % 2) * DV
                        rc = outpool.tile([128, 1], f32, tag="rc", name="rc")
                        nc.vector.reciprocal(out=rc, in_=ot[:, col + D:col + DV])
                        osb = outpool.tile([128, D], f32, tag="osb", name="osb")
                        nc.vector.tensor_scalar_mul(osb, ot[:, col:col + D], rc)
                        r0 = s * QSUP + c * 128
                        nc.sync.dma_start(
                            out=o_part[r0:r0 + 128, h * D:(h + 1) * D], in_=osb)

            for h in range(QH):
                for s in range(NSUP):
                    o01 = opool.tile([128, 2 * DV], f32, tag="o01", name="o01")
                    o23 = opool.tile([128, 2 * DV], f32, tag="o23", name="o23")
                    nk = 4 * s + 4
                    qoff = h * T + s * QSUP
                    for b0 in range(0, nk, KBATCH):
                        batch = list(range(b0, min(b0 + KBATCH, nk)))
                        nb = len(batch)
                        st = stpool.tile([128, nb * QSUP], f32, tag="st", name="st")
                        for j, ki in enumerate(batch):
                            nc.tensor.matmul(
                                st[:, j * QSUP:(j + 1) * QSUP],
                                lhsT=kT_sb[:, ki * 128:(ki + 1) * 128],
                                rhs=qT_sb[:, qoff:qoff + QSUP],
                                start=True, stop=True,
                            )
                        pt = wpool.tile([128, nb * QSUP], f16, tag="pt", name="pt")
                        nc.scalar.activation(out=pt, in_=st, func=EXP)
                        for j, ki in enumerate(batch):
                            o = ki - 4 * s
                            if o >= 0:
                                sl = pt[:, j * QSUP:(j + 1) * QSUP]
                                nc.vector.tensor_mul(
                                    sl, sl, mk_sb[:, o * QSUP:(o + 1) * QSUP])
                        flush()
                        pending[0] = (batch, pt, o01, o23, s, nk, h,
                                      b0 + KBATCH >= nk)
            flush()

            # Cache update: pure DRAM->DRAM copies, chunked <=1024 rows.
            for (dst0, src0, n, from_new) in plan:
                s_t = src if from_new else cin
                for off in range(0, n, 1024):
                    m = min(1024, n - off)
                    nc.sync.dma_start(
                        out=cout[dst0 + off:dst0 + off + m, :],
                        in_=s_t[src0 + off:src0 + off + m, :])

    nc.compile()
    return nc


def _get_program(plan):
    if plan not in _PROG_CACHE:
        _PROG_CACHE[plan] = _build_program(plan)
    return _PROG_CACHE[plan]


def _make_masks():
    tk = np.arange(128)[:, None] // BL          # [128,1] 0..3
    ql = np.arange(QSUP)[None, :] // BL         # [1,512] 0..15
    cols = []
    for o in range(4):
        cols.append(((4 * o + tk) <= ql).astype(np.float16))
    return np.concatenate(cols, axis=1)         # [128, 2048]


def kernel(q, k, v, k_cache, v_cache, slot_mapping, block_length):
    global LAST_RESULT
    from concourse.bass_utils import run_bass_kernel_spmd

    q = np.ascontiguousarray(np.asarray(q, dtype=np.float32))
    k = np.ascontiguousarray(np.asarray(k, dtype=np.float32))
    v = np.ascontiguousarray(np.asarray(v, dtype=np.float32))
    k_cache = np.ascontiguousarray(np.asarray(k_cache, dtype=np.float32))
    v_cache = np.ascontiguousarray(np.asarray(v_cache, dtype=np.float32))
    sm = np.asarray(slot_mapping).astype(np.int64)
    assert int(block_length) == BL
    assert q.shape == (T, H * D) and k.shape == (T, HKV * D)

    plan = _plan_cache(sm)
    nc = _get_program(plan)

    qh = q.reshape(T, H, D)
    kh = k.reshape(T, HKV, D)
    vh = v.reshape(T, HKV, D)
    kch = k_cache.reshape(NUM_SLOTS, HKV, D)
    vch = v_cache.reshape(NUM_SLOTS, HKV, D)
    mk = _make_masks()

    in_maps = []
    for c in range(NCORES):
        g = c // 2
        qTc = np.ascontiguousarray(
            (qh[:, 2 * c:2 * c + 2, :] * SCALE).transpose(1, 2, 0),
            dtype=np.float32)                       # [2,128,T]
        kTc = np.ascontiguousarray(kh[:, g, :].T)   # [128,T]
        vpc = np.ones((T, DV), np.float16)
        vpc[:, :D] = vh[:, g, :].astype(np.float16)
        vpc = np.ascontiguousarray(
            vpc.reshape(NKT, 128, DV).transpose(1, 0, 2).reshape(128, NKT * DV))
        if c % 2 == 0:
            cin = np.ascontiguousarray(kch[:, g, :])
            srcr = np.ascontiguousarray(kh[:, g, :])
        else:
            cin = np.ascontiguousarray(vch[:, g, :])
            srcr = np.ascontiguousarray(vh[:, g, :])
        in_maps.append({"qT": qTc, "kT": kTc, "vp": vpc, "mk": mk,
                        "cin": cin, "src": srcr})

    res = run_bass_kernel_spmd(nc, in_maps, list(range(NCORES)),
                               trace=bool(os.environ.get("KNL_TRACE")))
    LAST_RESULT = res

    o = np.concatenate([res.results[c]["o_part"] for c in range(NCORES)],
                       axis=1)
    kc = np.empty((NUM_SLOTS, HKV * D), np.float32)
    vc = np.empty((NUM_SLOTS, HKV * D), np.float32)
    for c in range(NCORES):
        g = c // 2
        dst = kc if c % 2 == 0 else vc
        dst[:, g * D:(g + 1) * D] = res.results[c]["cout"]
    return o, kc, vc
